# revision 18
# baseline (speedup 1.0000x reference)
"""Trainium2 Bass kernel for nn_CSCLoss: multi-scale bilinear point-sampling
cosine-consistency loss.

loss = 1 - mean_{pairs,(b,n)} <normalize(sample(feat_i, p_bn)), normalize(sample(feat_j, p_bn))>

Sharding: data-parallel over batch — 32 images -> 8 cores x 4 images; the
host sums the 8 per-core partial sums and applies the 1 - total/count
epilogue (the all-reduce of the sharding hint, done on 8 scalars).

Per-core dataflow (HBM-bandwidth-bound, ~22 MB/core):
 - Each level streams in DRAM-FLAT layout: partition p holds a contiguous
   1/128 slice of the level = 8 consecutive (b, c) rows, i.e.
   b = p//32, channels c = 8*(p%32)..8*(p%32)+7 each of H*W pixels.
   Both halves of the free dim go on the two HWDGE rings -> every
   descriptor is a big contiguous run (64 KB for lvl0), so the stream
   runs at HBM line rate with no small-descriptor trickle.
 - ONE ap_gather per level pulls all 4 bilinear corners (k) for all 32
   points (s) and all 8 channel offsets (o): out col j = s*32 + o*4 + k,
   idx = y0*W + x0 + dk(k) + o*H*W  (image-local; the partition encodes
   the image).  ap_gather's per-16-partition-group index blocks give each
   image's two groups their own point indices.  num_elems = 32768 for
   lvl0 = exactly the int16/2^15 ap_gather limit.
 - Index and corner-weight rows are computed on partition 0 by DVE from
   `boxes`, bounced through DRAM, and replicated into the [128, *] SBUF
   tiles with 0-stride SWDGE DMAs.  All staging for all levels runs
   before the first gather so the GpSimd FIFO never blocks staging.
 - Per-point channel sums: V_i*V_j products on DVE, then a matmul with a
   static [128, 4] image-selector contracts partitions -> PSUM [4, (s,o)],
   DVE reduces o -> [4, 32] per-point sums.  Cosine epilogue on [4, 32],
   final cross-image sum via a [4,1]x[4,1] matmul, one [1,1] DMA out.
"""

import sys
from contextlib import ExitStack

import numpy as np

if "/opt/trn_rl_repo" not in sys.path:
    sys.path.insert(0, "/opt/trn_rl_repo")

B, N, C = 32, 32, 256
LEVELS = [(64, 64), (32, 32), (16, 16)]  # (H, W)
N_CORES = 8
BL = B // N_CORES          # images per core
NPTS = BL * N              # 128 points per core
PAIRS = [(0, 1), (0, 2), (1, 2)]
EPS = 1e-12

_CACHE = {}


def _build_program():
    from concourse import bacc, bass, mybir, tile, library_config

    dt = mybir.dt
    AL = mybir.AluOpType

    nc = bacc.Bacc("TRN2", target_bir_lowering=False, debug=False)

    feats = [
        nc.dram_tensor(f"feat{i}", [BL, C, H, W], dt.float32, kind="ExternalInput")
        for i, (H, W) in enumerate(LEVELS)
    ]
    boxes = nc.dram_tensor("boxes", [BL, N, 4], dt.float32, kind="ExternalInput")
    out = nc.dram_tensor("out", [1, 1], dt.float32, kind="ExternalOutput")

    with tile.TileContext(nc) as tc, ExitStack() as ctx:
        pool = ctx.enter_context(tc.tile_pool(name="sbuf", bufs=1))
        pa = ctx.enter_context(tc.tile_pool(name="pa", bufs=1))
        pstream = ctx.enter_context(tc.tile_pool(name="stream", bufs=1))
        pwork = ctx.enter_context(tc.tile_pool(name="work", bufs=1))
        ppsum = ctx.enter_context(tc.tile_pool(name="psum", bufs=1, space="PSUM"))
        pdram = ctx.enter_context(tc.tile_pool(name="dram", bufs=1, space="DRAM"))

        nc.gpsimd.load_library(library_config.ap_gather)

        # ---- static setup ----
        # P4sel[p, m] = 1.0 iff p//32 == m  (colsum lhsT: contract partitions
        # into per-image rows);  P4T[m, p] = its transpose (replication lhsT).
        p4sel = pool.tile([128, 4], dt.float32)
        nc.vector.memset(p4sel[:], 0.0)
        for m in range(4):
            nc.vector.memset(p4sel[32 * m:32 * (m + 1), m:m + 1], 1.0)
        # p4t[m, col] = 1.0 iff col//32 == m, built partition-base-0 legal:
        # (col + 96*m)>>5 & 3 == (col//32 - m) mod 4 == 0  <=>  col//32 == m.
        p4i = pool.tile([4, 128], dt.int32, name="p4i")
        nc.gpsimd.iota(p4i[:], pattern=[[1, 128]], base=0, channel_multiplier=96)
        nc.vector.tensor_scalar(
            out=p4i[:], in0=p4i[:], scalar1=5, scalar2=3,
            op0=AL.arith_shift_right, op1=AL.bitwise_and,
        )
        p4t = pool.tile([4, 128], dt.float32)
        nc.vector.tensor_scalar(
            out=p4t[:], in0=p4i[:], scalar1=0, scalar2=None, op0=AL.is_equal,
        )
        ones4 = pool.tile([4, 1], dt.float32)
        nc.vector.memset(ones4[:], 1.0)

        # ---- boxes load first on the sync ring (phase A needs it) ----
        bxr = pool.tile([1, BL * N * 4], dt.float32)  # [1, 512] flat boxes
        nc.sync.dma_start(
            out=bxr[:].rearrange("o (a f) -> o a f", a=BL),
            in_=boxes.rearrange("b n c -> b (n c)"),
        )

        # ---- feature streams: DRAM-flat [128, E], halves on the 2 rings ----
        T_tiles = []
        for li, (H, W) in enumerate(LEVELS):
            E = BL * C * H * W // 128  # elems per partition (8 rows of H*W)
            fflat = feats[li].rearrange("b c h w -> (b c h w)").rearrange(
                "(p q) -> p q", p=128
            )
            T = pstream.tile([128, E], dt.float32, name=f"T{li}")
            nc.sync.dma_start(out=T[:, 0:E // 2], in_=fflat[:, 0:E // 2])
            nc.scalar.dma_start(out=T[:, E // 2:E], in_=fflat[:, E // 2:E])
            T_tiles.append(T)

        # ---- Phase A (per level): point math on partition 0 + staging ----
        bxv = bxr[:].rearrange("o (j c) -> o j c", c=4)
        cx = bxv[:, :, 0]  # [1, 128] stride 4, point (b, s) at col b*32+s
        cy = bxv[:, :, 1]

        def axis_prep(coord, Eax, ax):
            """pixel coord p=clip(c*(E-1),0,E-1); e0=clamp(floor(p),0,E-2);
            w=p-e0. floor via 16.16 fixed point (exact *2^16; conversion
            error <=2^-16 absorbed by the lerp weight)."""
            pf = pa.tile([1, NPTS], dt.float32, name=f"pf{ax}", tag="pf")
            nc.vector.tensor_scalar(
                out=pf[:], in0=coord, scalar1=float(Eax - 1), scalar2=0.0,
                op0=AL.mult, op1=AL.max,
            )
            nc.vector.tensor_scalar_min(out=pf[:], in0=pf[:], scalar1=float(Eax - 1))
            pxs = pa.tile([1, NPTS], dt.float32, name=f"pxs{ax}", tag="pxs")
            nc.vector.tensor_scalar(
                out=pxs[:], in0=pf[:], scalar1=65536.0, scalar2=None, op0=AL.mult,
            )
            ifx = pa.tile([1, NPTS], dt.int32, name=f"ifx{ax}", tag="ifx")
            nc.vector.tensor_copy(out=ifx[:], in_=pxs[:])
            x0i = pa.tile([1, NPTS], dt.int32, name=f"x0i{ax}", tag="x0i")
            nc.vector.tensor_scalar(
                out=x0i[:], in0=ifx[:], scalar1=16, scalar2=None,
                op0=AL.arith_shift_right,
            )
            e0 = pa.tile([1, NPTS], dt.float32, name=f"e0{ax}", tag=f"e0{ax}")
            nc.vector.tensor_copy(out=e0[:], in_=x0i[:])
            nc.vector.tensor_scalar_min(out=e0[:], in0=e0[:], scalar1=float(Eax - 2))
            we = pa.tile([1, NPTS], dt.float32, name=f"we{ax}", tag=f"we{ax}")
            nc.vector.tensor_tensor(out=we[:], in0=pf[:], in1=e0[:], op=AL.subtract)
            return e0, we

        widxs, wbs = [], []
        for li, (H, W) in enumerate(LEVELS):
            HW = H * W
            x0f, wx = axis_prep(cx, W, "x")
            y0f, wy = axis_prep(cy, H, "y")

            # basef[(b,s)] = y0*W + x0  (image-local: the partition holds b)
            basef = pa.tile([1, NPTS], dt.float32, name="basef", tag="basef")
            nc.vector.tensor_scalar(
                out=basef[:], in0=y0f[:], scalar1=float(W), scalar2=None,
                op0=AL.mult,
            )
            nc.vector.tensor_tensor(
                out=basef[:], in0=basef[:], in1=x0f[:], op=AL.add
            )

            # wrapped index rows: gather-out col j = s*32 + o*4 + k reads the
            # group-local index at (partition r=j%16, col cb=j//16=s*2+half),
            # with o = half*4 + r//4, k = r%4.
            # srow_all[1, 4096] holds the 4 images' [16, 64] blocks stacked:
            # flat col = b*1024 + r*64 + s*2 + half;
            # value = basef[b,s] + dk(k) + o*HW, dk = (k//2)*W + k%2.
            srow = pa.tile([1, 4 * 16 * 64], dt.float32, name="srow", tag="srow")
            srow_v = srow[:].rearrange(
                "o (b r s h) -> o b r s h", b=BL, r=16, s=32
            )
            basef_v = basef[:].rearrange("o (b s) -> o b s", b=BL)
            for r in range(16):
                for half in range(2):
                    k = r % 4
                    o = half * 4 + r // 4
                    dk = float((k // 2) * W + (k % 2) + o * HW)
                    nc.vector.tensor_scalar(
                        out=srow_v[:, :, r, :, half], in0=basef_v[:],
                        scalar1=dk, scalar2=None, op0=AL.add,
                    )
            # write the wrapped rows to DRAM PRE-DUPLICATED ([128, 64]: each
            # image's [16, 64] block twice), then ONE plain contiguous-
            # partition read back.  (0-stride broadcast DMAs and multi-level
            # partition APs on the SBUF side both corrupt the transfer —
            # verified on HW.)
            sidx = pdram.tile([128, 64], dt.float32, name=f"sidx{li}")
            sidx_v = sidx[:].rearrange("(b d r) c -> b d r c", b=BL, d=2)
            srow_in = srow[:].rearrange("o (b r c) -> o b r c", b=BL, r=16)
            nc.gpsimd.dma_start(out=sidx_v[:, 0], in_=srow_in)
            nc.gpsimd.dma_start(out=sidx_v[:, 1], in_=srow_in)
            widx_f = pool.tile([128, 64], dt.float32, name=f"widxf{li}", tag="widxf")
            nc.gpsimd.dma_start(out=widx_f[:], in_=sidx[:])
            widx = pool.tile([128, 64], dt.int16, name=f"widx{li}")
            nc.vector.tensor_copy(out=widx[:], in_=widx_f[:])
            widxs.append(widx)

            # corner weights w(b, s, k), k = yi*2 + xi
            w1x = pa.tile([1, NPTS], dt.float32, name="w1x", tag="w1x")
            nc.vector.tensor_scalar(
                out=w1x[:], in0=wx[:], scalar1=-1.0, scalar2=1.0,
                op0=AL.mult, op1=AL.add,
            )
            w1y = pa.tile([1, NPTS], dt.float32, name="w1y", tag="w1y")
            nc.vector.tensor_scalar(
                out=w1y[:], in0=wy[:], scalar1=-1.0, scalar2=1.0,
                op0=AL.mult, op1=AL.add,
            )
            wkt = pa.tile([1, 4 * NPTS], dt.float32, name="wkt", tag="wkt")
            for k, (wyt, wxt) in enumerate(
                [(w1y, w1x), (w1y, wx), (wy, w1x), (wy, wx)]
            ):
                nc.vector.tensor_tensor(
                    out=wkt[:, k * NPTS:(k + 1) * NPTS],
                    in0=wyt[:], in1=wxt[:], op=AL.mult,
                )
            # wrow[(b, s, k)] <- wkt[(k, b, s)]
            wrow = pa.tile([1, NPTS * 4], dt.float32, name="wrow", tag="srow")
            wkt_v = wkt[:].rearrange("o (k b s) -> o k b s", k=4, b=BL)
            wrow_v = wrow[:].rearrange("o (b s k) -> o b s k", b=BL, k=4)
            for b in range(BL):
                nc.vector.tensor_copy(
                    out=wrow_v[:, b],
                    in_=wkt_v[:, :, b].rearrange("o k s -> o s k"),
                )
            wsk = pdram.tile([BL, 128], dt.float32, name=f"wsk{li}")
            nc.gpsimd.dma_start(
                out=wsk[:], in_=wrow[:].rearrange("o (b c) -> o b c", b=BL),
            )
            # replicate each image's [1, (s k)] row to its 32 partitions via
            # the P4T matmul: wb[p, c] = wsk[p//32, c].
            s4f = pa.tile([BL, 128], dt.float32, name="s4f", tag="s4f")
            nc.gpsimd.dma_start(out=s4f[:], in_=wsk[:])
            wb_ps = ppsum.tile([128, 128], dt.float32, name=f"wbps{li}", tag="wbps")
            nc.tensor.matmul(wb_ps[:], p4t[:], s4f[:], start=True, stop=True)
            wb = pool.tile([128, 128], dt.float32, name=f"wb{li}")
            nc.vector.tensor_copy(out=wb[:], in_=wb_ps[:])
            wbs.append(wb)

        # ---- gathers (one per level) + lerp ----
        V = [pool.tile([128, NPTS * 2], dt.float32, name=f"V{li}") for li in range(3)]
        for li, (H, W) in enumerate(LEVELS):
            HW = H * W
            E = BL * C * HW // 128
            og = pwork.tile([128, 1024], dt.float32, name=f"og{li}", tag="og")
            nc.gpsimd.ap_gather(
                out_ap=og[:], in_ap=T_tiles[li][:], idxs_ap=widxs[li][:],
                channels=128, num_elems=E, d=1, num_idxs=1024,
            )
            og_v = og[:].rearrange("c (s o k) -> c s o k", s=32, o=8)
            wb_v = wbs[li][:].rearrange("c (s k) -> c s k", s=32)
            for o in range(8):
                nc.vector.tensor_tensor(
                    out=og_v[:, :, o], in0=og_v[:, :, o], in1=wb_v[:], op=AL.mult,
                )
            # sum the 4 corners -> V[p, s*8 + o]
            nc.vector.tensor_reduce(
                out=V[li][:],
                in_=og[:].rearrange("c (n k) -> c n k", k=4),
                axis=mybir.AxisListType.X, op=AL.add,
            )

        # ---- per-point channel sums: partitions contract via P4sel matmul.
        # prod[p, (s,o)] -> psum[4, (s,o)] -> reduce o -> [4, 32] (b, s).
        def colsum(name, vi, vj):
            prod = pwork.tile([128, NPTS * 2], dt.float32, name=f"prod{name}", tag="og")
            nc.vector.tensor_tensor(out=prod[:], in0=vi[:], in1=vj[:], op=AL.mult)
            ps = ppsum.tile([4, NPTS * 2], dt.float32, name=name)
            nc.tensor.matmul(ps[:], p4sel[:], prod[:], start=True, stop=True)
            sb = pool.tile([4, 32], dt.float32, name=f"sb{name}")
            nc.vector.tensor_reduce(
                out=sb[:], in_=ps[:].rearrange("p (s o) -> p s o", o=8),
                axis=mybir.AxisListType.X, op=AL.add,
            )
            return sb

        ss = [colsum(f"ss{li}", V[li], V[li]) for li in range(3)]
        dots = {(i, j): colsum(f"d{i}{j}", V[i], V[j]) for i, j in PAIRS}

        # ---- cosine epilogue on [4, 32] ----
        rns = []
        for li in range(3):
            nrm = pool.tile([4, 32], dt.float32, name=f"nrm{li}")
            nc.scalar.sqrt(out=nrm[:], in_=ss[li][:])
            nc.vector.tensor_scalar_max(out=nrm[:], in0=nrm[:], scalar1=EPS)
            rn = pool.tile([4, 32], dt.float32, name=f"rn{li}")
            nc.vector.reciprocal(out=rn[:], in_=nrm[:])
            rns.append(rn)

        tot = pool.tile([4, 32], dt.float32)
        first = True
        for i, j in PAIRS:
            t = pool.tile([4, 32], dt.float32, name=f"t{i}{j}")
            nc.vector.tensor_tensor(
                out=t[:], in0=dots[(i, j)][:], in1=rns[i][:], op=AL.mult
            )
            nc.vector.tensor_tensor(out=t[:], in0=t[:], in1=rns[j][:], op=AL.mult)
            if first:
                nc.vector.tensor_copy(out=tot[:], in_=t[:])
                first = False
            else:
                nc.vector.tensor_tensor(out=tot[:], in0=tot[:], in1=t[:], op=AL.add)

        tot4 = pool.tile([4, 1], dt.float32)
        nc.vector.tensor_reduce(
            out=tot4[:], in_=tot[:], axis=mybir.AxisListType.X, op=AL.add
        )
        res_ps = ppsum.tile([1, 1], dt.float32, name="resps")
        nc.tensor.matmul(res_ps[:], tot4[:], ones4[:], start=True, stop=True)
        res = pool.tile([1, 1], dt.float32)
        nc.vector.tensor_copy(out=res[:], in_=res_ps[:])
        nc.sync.dma_start(out=out.ap(), in_=res[:])

    nc.compile()
    return nc


def _get_program():
    if "nc" not in _CACHE:
        _CACHE["nc"] = _build_program()
    return _CACHE["nc"]


def _run_device(feat0, feat1, feat2, boxes, **run_kwargs):
    """Shard inputs batch-wise over the 8 cores, run the SPMD program, and
    return the BassKernelResults (one {"out": [1,1]} per core)."""
    from concourse.bass_utils import run_bass_kernel_spmd

    nc = _get_program()

    feats = [
        np.ascontiguousarray(np.asarray(f, dtype=np.float32))
        for f in (feat0, feat1, feat2)
    ]
    boxes = np.ascontiguousarray(np.asarray(boxes, dtype=np.float32))

    in_maps = []
    for k in range(N_CORES):
        sl = slice(k * BL, (k + 1) * BL)
        in_maps.append(
            {
                "feat0": feats[0][sl],
                "feat1": feats[1][sl],
                "feat2": feats[2][sl],
                "boxes": boxes[sl],
            }
        )

    return run_bass_kernel_spmd(
        nc, in_maps, core_ids=list(range(N_CORES)), **run_kwargs
    )


def kernel(feat0, feat1, feat2, boxes):
    r = _run_device(feat0, feat1, feat2, boxes)
    total = np.float64(0.0)
    for m in r.results:
        total += np.float64(m["out"].reshape(-1)[0])

    count = B * N * len(PAIRS)
    avg = np.float32(total) / np.float32(count)
    loss = np.float32(1.0) - avg
    loss = np.nan_to_num(loss, nan=0.0, posinf=1.0, neginf=0.0)
    return np.array(np.clip(loss, 0.0, 2.0), dtype=np.float32)
